# revision 9
# baseline (speedup 1.0000x reference)
"""Trainium2 Bass kernel for the CloudCast composite loss.

Strategy (pure data parallel): B=8 samples, one sample per NeuronCore.
Each core streams its sample's four [768,768] fp32 maps from HBM once
(~9.4 MB), computes all per-sample reductions with fused
elementwise+accumulate ops spread across DVE/ACT/GPSIMD/PE, and resolves
the hard-negative-mining top-k threshold with an on-device binary search
over a strided subset of the masked focal map held in SBUF.  The host
combines the ~40 scalars per core (the "all-reduce" of scalar sums).

Math notes (t is exactly {0,1} for this loss):
  u = |t - p_clip|, s = u^2, f1 = s*ln(1-u)  (<= 0)
    focal = -(0.25 + 1.25 t) * f1        (covers both BCE branches, POS_W=2)
  fneg = f1 * [t < 0.5]                  (masked; top-k negatives = bottom-k fneg)
  top-k sum via threshold theta:  sum_{v<theta} v + (k - N(theta)) * theta
    which is second-order accurate in (theta - v_(k)).
  huber: hub/2 = mb^2 - mb + a,  a = |rl - ln(1+rs)|, mb = min(a, .5)
  w = (max(10p, rs) > 1) + 3*(rs >= 50)   (since 10p <= 10 < 50)
"""

import numpy as np

import concourse.bass as bass
import concourse.bacc as bacc
import concourse.tile as tile
import concourse.mybir as mybir
from concourse.bass_utils import run_bass_kernel_spmd

F32 = mybir.dt.float32
BF16 = mybir.dt.bfloat16
ALU = mybir.AluOpType
ACTF = mybir.ActivationFunctionType
AXX = mybir.AxisListType.X

B = 8
P = 128
F = 768 * 768 // P          # 4608
NPIX = P * F                # 589824
NCHUNK = 4
FC = F // NCHUNK            # 1152
EPS = 1e-6
NITER = 12
SUBSTRIDE = 16
NSUB = F // SUBSTRIDE       # 288
NOUT = 36

# output vector slots (after partition reduction)
SL_T, SL_D, SL_S, SL_P2, SL_F1, SL_FN, SL_W, SL_HW = 0, 4, 8, 12, 16, 20, 24, 28
SL_SS, SL_NN, SL_TH, SL_KK = 32, 33, 34, 35


def _trace_body(tc, out, prob, lab, rlg, rsp):
    nc = tc.nc
    with (
        tc.tile_pool(name="inp", bufs=2) as inp,
        tc.tile_pool(name="w32", bufs=2) as w32,
        tc.tile_pool(name="wbf", bufs=2) as wbf,
        tc.tile_pool(name="scr", bufs=2) as scr,
        tc.tile_pool(name="per", bufs=1) as per,
        tc.tile_pool(name="sml", bufs=2) as sml,
        tc.tile_pool(name="ps", bufs=2, space=bass.MemorySpace.PSUM) as psp,
    ):
        # persistent state
        fneg = per.tile([P, F], BF16)
        ones = per.tile([P, P], F32)
        nc.vector.memset(ones[:], 1.0)
        ones1 = per.tile([P, 1], F32)
        nc.vector.memset(ones1[:], 1.0)
        # one accumulator tile per quantity: avoids false WAW deps between
        # the big ops that carry the fused accumulations
        acc_t = per.tile([P, NCHUNK], F32)
        acc_d = per.tile([P, NCHUNK], F32)
        acc_s = per.tile([P, NCHUNK], F32)
        acc_p2 = per.tile([P, NCHUNK], F32)
        acc_f1 = per.tile([P, NCHUNK], F32)
        acc_fn = per.tile([P, NCHUNK], F32)
        acc_w = per.tile([P, NCHUNK], F32)
        acc_hw = per.tile([P, NCHUNK], F32)
        acc_ss = per.tile([P, 1], F32)
        acc_nn = per.tile([P, 1], F32)

        for i in range(NCHUNK):
            cs = bass.ts(i, FC)
            p = inp.tile([P, FC], F32, tag="p")
            nc.sync.dma_start(p[:], prob[:, cs])
            t = inp.tile([P, FC], F32, tag="t")
            nc.sync.dma_start(t[:], lab[:, cs])
            r1 = inp.tile([P, FC], F32, tag="r1")
            nc.sync.dma_start(r1[:], rlg[:, cs])
            r2 = inp.tile([P, FC], F32, tag="r2")
            nc.sync.dma_start(r2[:], rsp[:, cs])

            # ---- focal / tversky ----
            pc = w32.tile([P, FC], F32, tag="pc")
            nc.vector.tensor_scalar(pc[:], p[:], EPS, 1.0 - EPS, ALU.max, ALU.min)
            d = w32.tile([P, FC], F32, tag="d")
            nc.vector.scalar_tensor_tensor(
                d[:], pc[:], -1.0, t[:], ALU.mult, ALU.add,
                accum_out=acc_d[:, i : i + 1])
            tbf = wbf.tile([P, FC], BF16, tag="tbf")
            nc.vector.tensor_scalar(
                tbf[:], t[:], 1.0, None, ALU.mult, ALU.add,
                accum_out=acc_t[:, i : i + 1])
            u = w32.tile([P, FC], F32, tag="u")
            nc.vector.scalar_tensor_tensor(
                u[:], d[:], -1.0, d[:], ALU.mult, ALU.max)
            s = wbf.tile([P, FC], BF16, tag="s")
            nc.scalar.activation(
                s[:], d[:], ACTF.Square, accum_out=acc_s[:, i : i + 1])
            lg = wbf.tile([P, FC], BF16, tag="lg")
            nc.scalar.activation(lg[:], u[:], ACTF.Ln, bias=1.0, scale=-1.0)
            f1 = wbf.tile([P, FC], BF16, tag="f1")
            nc.vector.scalar_tensor_tensor(
                f1[:], s[:], 1.0, lg[:], ALU.mult, ALU.mult)
            nc.vector.scalar_tensor_tensor(
                fneg[:, cs], tbf[:], 0.5, f1[:], ALU.is_lt, ALU.mult,
                accum_out=acc_fn[:, i : i + 1])
            # sum the *quantized* f1 tile so (sf1 - sfn) is elementwise exact
            fsc = scr.tile([P, FC], BF16, tag="fsc")
            nc.vector.tensor_scalar(
                fsc[:], f1[:], 1.0, None, ALU.mult, ALU.add,
                accum_out=acc_f1[:, i : i + 1])
            ssc = scr.tile([P, FC], BF16, tag="ssc")
            nc.scalar.activation(
                ssc[:], pc[:], ACTF.Square, accum_out=acc_p2[:, i : i + 1])

            # ---- huber + gating ----
            rlt = wbf.tile([P, FC], BF16, tag="rlt")
            nc.scalar.activation(rlt[:], r2[:], ACTF.Ln, bias=1.0, scale=1.0)
            rlb = wbf.tile([P, FC], BF16, tag="rlb")
            nc.gpsimd.tensor_copy(rlb[:], r1[:])
            dh = wbf.tile([P, FC], BF16, tag="dh")
            nc.vector.scalar_tensor_tensor(
                dh[:], rlt[:], -1.0, rlb[:], ALU.mult, ALU.add)
            a = wbf.tile([P, FC], BF16, tag="a")
            nc.vector.scalar_tensor_tensor(
                a[:], dh[:], -1.0, dh[:], ALU.mult, ALU.max)
            mb = wbf.tile([P, FC], BF16, tag="mb")
            nc.vector.tensor_scalar(mb[:], a[:], 0.5, None, ALU.min)
            w1 = wbf.tile([P, FC], BF16, tag="w1")
            nc.vector.tensor_scalar(w1[:], mb[:], -1.0, None, ALU.add)
            zz = wbf.tile([P, FC], BF16, tag="zz")
            nc.vector.scalar_tensor_tensor(
                zz[:], w1[:], 1.0, mb[:], ALU.mult, ALU.mult)
            hc = wbf.tile([P, FC], BF16, tag="hc")
            nc.vector.scalar_tensor_tensor(
                hc[:], zz[:], 1.0, a[:], ALU.mult, ALU.add)
            zm = wbf.tile([P, FC], BF16, tag="zm")
            nc.vector.scalar_tensor_tensor(
                zm[:], p[:], 10.0, r2[:], ALU.mult, ALU.max)
            za = wbf.tile([P, FC], BF16, tag="za")
            nc.vector.tensor_scalar(za[:], zm[:], 1.0, None, ALU.is_gt)
            zb = wbf.tile([P, FC], BF16, tag="zb")
            nc.vector.tensor_scalar(zb[:], r2[:], 50.0, 3.0, ALU.is_ge, ALU.mult)
            w = wbf.tile([P, FC], BF16, tag="w")
            nc.vector.scalar_tensor_tensor(
                w[:], za[:], 1.0, zb[:], ALU.mult, ALU.add,
                accum_out=acc_w[:, i : i + 1])
            hsc = scr.tile([P, FC], BF16, tag="hsc")
            nc.vector.scalar_tensor_tensor(
                hsc[:], hc[:], 1.0, w[:], ALU.mult, ALU.mult,
                accum_out=acc_hw[:, i : i + 1])

        # ---- n_pos -> subset top-k target ----
        tsum = sml.tile([P, 1], F32, tag="tsum")
        nc.vector.tensor_reduce(tsum[:], acc_t[:], AXX, ALU.add)
        npbc = psp.tile([P, 1], F32, tag="npbc")
        nc.tensor.matmul(npbc[:], ones[:], tsum[:], start=True, stop=True)
        npv = sml.tile([P, 1], F32, tag="npv")
        nc.scalar.activation(npv[:], npbc[:], ACTF.Identity)
        ka = sml.tile([P, 1], F32, tag="ka")
        nc.vector.tensor_scalar(ka[:], npv[:], 10.0 / SUBSTRIDE, None, ALU.mult)
        kb = sml.tile([P, 1], F32, tag="kb")
        nc.vector.tensor_scalar(
            kb[:], npv[:], -1.0 / SUBSTRIDE, float(NPIX // SUBSTRIDE),
            ALU.mult, ALU.add)
        kk = sml.tile([P, 1], F32, tag="kk")
        nc.vector.scalar_tensor_tensor(kk[:], ka[:], 1.0, kb[:], ALU.mult, ALU.min)

        # strided subset of fneg (every 16th element)
        sub = per.tile([P, NSUB], BF16)
        fview = fneg[:].rearrange("p (n s) -> p n s", s=SUBSTRIDE)[:, :, 0:1]
        nc.vector.tensor_copy(sub[:].unsqueeze(-1), fview)

        # ---- binary search for theta (in f1 units, negative) ----
        th = sml.tile([P, 1], F32, tag="th")
        nc.vector.memset(th[:], -7.0)
        delta = 3.5
        for _ in range(NITER):
            csc = sml.tile([P, NSUB], BF16, tag="csc")
            cnt = sml.tile([P, 1], F32, tag="cnt")
            nc.vector.tensor_scalar(
                csc[:], sub[:], th[:], None, ALU.is_lt, ALU.add,
                accum_out=cnt[:])
            cbc = psp.tile([P, 1], F32, tag="cbc")
            nc.tensor.matmul(cbc[:], ones[:], cnt[:], start=True, stop=True)
            sg = sml.tile([P, 1], F32, tag="sg")
            nc.scalar.activation(sg[:], cbc[:], ACTF.Sign, bias=kk[:], scale=-1.0)
            th2 = sml.tile([P, 1], F32, tag="th")
            nc.scalar.activation(th2[:], sg[:], ACTF.Identity, bias=th[:], scale=delta)
            th = th2
            delta *= 0.5

        # ---- exact masked count + sum at theta over the full map ----
        nsc = scr.tile([P, F], BF16, tag="nsc")
        nc.vector.tensor_scalar(
            nsc[:], fneg[:], th[:], None, ALU.is_lt, ALU.add,
            accum_out=acc_nn[:])
        ssc2 = scr.tile([P, F], BF16, tag="nsc")
        nc.vector.scalar_tensor_tensor(
            ssc2[:], fneg[:], th[:], fneg[:], ALU.is_lt, ALU.mult,
            accum_out=acc_ss[:])

        # ---- pack everything into out[1, NOUT] via ones-matmuls ----
        fin = psp.tile([1, NOUT], F32, tag="fin")
        nc.tensor.matmul(fin[:, SL_T:SL_T + 4], ones1[:], acc_t[:], start=True, stop=True)
        nc.tensor.matmul(fin[:, SL_D:SL_D + 4], ones1[:], acc_d[:], start=True, stop=True)
        nc.tensor.matmul(fin[:, SL_S:SL_S + 4], ones1[:], acc_s[:], start=True, stop=True)
        nc.tensor.matmul(fin[:, SL_P2:SL_P2 + 4], ones1[:], acc_p2[:], start=True, stop=True)
        nc.tensor.matmul(fin[:, SL_F1:SL_F1 + 4], ones1[:], acc_f1[:], start=True, stop=True)
        nc.tensor.matmul(fin[:, SL_FN:SL_FN + 4], ones1[:], acc_fn[:], start=True, stop=True)
        nc.tensor.matmul(fin[:, SL_W:SL_W + 4], ones1[:], acc_w[:], start=True, stop=True)
        nc.tensor.matmul(fin[:, SL_HW:SL_HW + 4], ones1[:], acc_hw[:], start=True, stop=True)
        nc.tensor.matmul(fin[:, SL_SS:SL_SS + 1], ones1[:], acc_ss[:], start=True, stop=True)
        nc.tensor.matmul(fin[:, SL_NN:SL_NN + 1], ones1[:], acc_nn[:], start=True, stop=True)
        nc.tensor.matmul(fin[:, SL_TH:SL_TH + 1], ones1[:], th[:], start=True, stop=True)
        nc.tensor.matmul(fin[:, SL_KK:SL_KK + 1], ones1[:], kk[:], start=True, stop=True)

        osb = sml.tile([1, NOUT], F32, tag="osb")
        nc.scalar.activation(osb[:], fin[:], ACTF.Identity)
        nc.sync.dma_start(out[:, :], osb[:])


def build_nc():
    nc = bacc.Bacc(
        "TRN2", target_bir_lowering=False, debug=False,
        enable_asserts=True, num_devices=B)
    prob = nc.dram_tensor("prob", [P, F], F32, kind="ExternalInput").ap()
    lab = nc.dram_tensor("lab", [P, F], F32, kind="ExternalInput").ap()
    rlg = nc.dram_tensor("rlg", [P, F], F32, kind="ExternalInput").ap()
    rsp = nc.dram_tensor("rsp", [P, F], F32, kind="ExternalInput").ap()
    out = nc.dram_tensor("out", [1, NOUT], F32, kind="ExternalOutput").ap()
    with tile.TileContext(nc) as tc:
        _trace_body(tc, out, prob, lab, rlg, rsp)
    nc.compile()
    return nc


_NC = None


def _get_nc():
    global _NC
    if _NC is None:
        _NC = build_nc()
    return _NC


def make_in_maps(prob_map, label_map, rain_logit, rain_spatial_true):
    maps = []
    for b in range(B):
        maps.append({
            "prob": np.ascontiguousarray(prob_map[b].reshape(P, F), dtype=np.float32),
            "lab": np.ascontiguousarray(label_map[b].reshape(P, F), dtype=np.float32),
            "rlg": np.ascontiguousarray(rain_logit[b].reshape(P, F), dtype=np.float32),
            "rsp": np.ascontiguousarray(rain_spatial_true[b].reshape(P, F), dtype=np.float32),
        })
    return maps


def _host_focal_sample(prob, lab, b):
    """Exact (float64) reference focal for one sample - slow fallback."""
    p = np.clip(prob.reshape(-1).astype(np.float64), EPS, 1.0 - EPS)
    t = lab.reshape(-1).astype(np.float64)
    bce = -(2.0 * t * np.log(p) + (1.0 - t) * np.log1p(-p))
    pos = t >= 0.5
    p_t = np.where(pos, p, 1.0 - p)
    a_t = np.where(pos, 0.75, 0.25)
    focal = a_t * (1.0 - p_t) ** 2 * bce
    n_pos = int(pos.sum())
    n_neg = focal.size - n_pos
    if n_pos > 0:
        k = min(10 * n_pos, n_neg)
        negf = focal[~pos]
        top = np.partition(negf, negf.size - k)[negf.size - k:].sum() if k > 0 else 0.0
        return (focal[pos].sum() + top) / max(n_pos + k, 1)
    import jax
    with jax.default_device(jax.devices("cpu")[0]):
        rs = np.asarray(jax.random.uniform(jax.random.key(42), (B, focal.size)))[b]
    order = np.argsort(np.where(pos, np.inf, rs), kind="stable")
    n_s = max(n_neg // 100, 1)
    return focal[order[:n_s]].sum() / n_s


def combine(vecs, prob_map, rain_logit, pred_phys, label_map,
            rain_spatial_true, phys_targets, phys_mu, phys_std):
    fls, tvs = [], []
    reg_num = 0.0
    reg_den = 0.0
    for b in range(B):
        v = vecs[b]
        st = v[SL_T:SL_T + 4].sum()
        sd = v[SL_D:SL_D + 4].sum()
        ss = v[SL_S:SL_S + 4].sum()
        sp2 = v[SL_P2:SL_P2 + 4].sum()
        sf1 = v[SL_F1:SL_F1 + 4].sum()
        sfn = v[SL_FN:SL_FN + 4].sum()
        sw = v[SL_W:SL_W + 4].sum()
        shw = v[SL_HW:SL_HW + 4].sum()
        S, Ncnt = v[SL_SS], v[SL_NN]
        th = v[SL_TH] / P
        n_pos = int(round(st))
        spc = st - sd
        tp = (st + sp2 - ss) / 2.0
        fp = spc - tp
        fn = st - tp
        tvs.append(1.0 - (tp + 1.0) / (tp + 0.3 * fp + 0.7 * fn + 1.0))
        n_neg = NPIX - n_pos
        k = min(10 * n_pos, n_neg)
        ok = n_pos > 0 and k >= 1600 and abs(Ncnt - k) <= max(64.0, 0.02 * k)
        if ok:
            top_f1 = S + (k - Ncnt) * th
            pos_f1 = sf1 - sfn
            fls.append((-1.5 * pos_f1 - 0.25 * top_f1) / max(n_pos + k, 1))
        else:
            fls.append(_host_focal_sample(prob_map[b], label_map[b], b))
        reg_num += 2.0 * shw
        reg_den += sw
    fl = float(np.mean(fls))
    tv = float(np.mean(tvs))
    reg = reg_num / max(reg_den, 1.0)
    tgt = np.nan_to_num(
        (phys_targets.astype(np.float64) - phys_mu.astype(np.float64))
        / (phys_std.astype(np.float64) + 1e-6))
    aux = float(np.mean((pred_phys.astype(np.float64) - tgt) ** 2))
    total = fl + 0.5 * tv + 1.0 * reg + 0.1 * aux
    f = np.float32
    return (f(total), f(fl), f(tv), f(reg), f(aux))


def kernel(prob_map, rain_logit, pred_phys, label_map, rain_max_true,
           rain_spatial_true, phys_targets, phys_mu, phys_std):
    nc = _get_nc()
    in_maps = make_in_maps(prob_map, label_map, rain_logit, rain_spatial_true)
    res = run_bass_kernel_spmd(nc, in_maps, core_ids=list(range(B)))
    vecs = [np.asarray(res.results[b]["out"]).reshape(-1).astype(np.float64)
            for b in range(B)]
    return combine(vecs, prob_map, rain_logit, pred_phys, label_map,
                   rain_spatial_true, phys_targets, phys_mu, phys_std)


# revision 10
# speedup vs baseline: 1.6007x; 1.6007x over previous
"""Trainium2 Bass kernel for the CloudCast composite loss.

Strategy (pure data parallel): B=8 samples, one sample per NeuronCore.
Each core streams its sample's four [768,768] fp32 maps from HBM once
(~9.4 MB), computes all per-sample reductions with fused
elementwise+accumulate ops spread across DVE/ACT/GPSIMD/PE, and resolves
the hard-negative-mining top-k threshold with an on-device binary search
over a strided subset of the masked focal map held in SBUF.  The host
combines the ~40 scalars per core (the "all-reduce" of scalar sums).

Math notes (t is exactly {0,1} for this loss):
  u = |t - p_clip|, s = u^2, f1 = s*ln(1-u)  (<= 0)
    focal = -(0.25 + 1.25 t) * f1        (covers both BCE branches, POS_W=2)
  fneg = f1 * [t < 0.5]                  (masked; top-k negatives = bottom-k fneg)
  top-k sum via threshold theta:  sum_{v<theta} v + (k - N(theta)) * theta
    which is second-order accurate in (theta - v_(k)).
  huber: hub/2 = mb^2 - mb + a,  a = |rl - ln(1+rs)|, mb = min(a, .5)
  w = (max(10p, rs) > 1) + 3*(rs >= 50)   (since 10p <= 10 < 50)
"""

import numpy as np

import concourse.bass as bass
import concourse.bacc as bacc
import concourse.tile as tile
import concourse.mybir as mybir
from concourse.bass_utils import run_bass_kernel_spmd

F32 = mybir.dt.float32
BF16 = mybir.dt.bfloat16
ALU = mybir.AluOpType
ACTF = mybir.ActivationFunctionType
AXX = mybir.AxisListType.X

B = 8
P = 128
F = 768 * 768 // P          # 4608
NPIX = P * F                # 589824
NCHUNK = 4
FC = F // NCHUNK            # 1152
EPS = 1e-6
NITER = 9
SUBSTRIDE = 16
NSUB = F // SUBSTRIDE       # 288
NOUT = 36

# output vector slots (after partition reduction)
SL_T, SL_D, SL_S, SL_P2, SL_F1, SL_FN, SL_W, SL_HW = 0, 4, 8, 12, 16, 20, 24, 28
SL_SS, SL_NN, SL_TH, SL_KK = 32, 33, 34, 35


def _trace_body(tc, out, prob, lab, rlg, rsp):
    nc = tc.nc
    with (
        tc.tile_pool(name="inp", bufs=2) as inp,
        tc.tile_pool(name="w32", bufs=2) as w32,
        tc.tile_pool(name="wbf", bufs=2) as wbf,
        tc.tile_pool(name="scr", bufs=2) as scr,
        tc.tile_pool(name="per", bufs=1) as per,
        tc.tile_pool(name="sml", bufs=2) as sml,
        tc.tile_pool(name="ps", bufs=2, space=bass.MemorySpace.PSUM) as psp,
    ):
        # persistent state
        fneg = per.tile([P, F], BF16)
        ones = per.tile([P, P], F32)
        nc.vector.memset(ones[:], 1.0)
        ones1 = per.tile([P, 1], F32)
        nc.vector.memset(ones1[:], 1.0)
        # one accumulator tile per quantity: avoids false WAW deps between
        # the big ops that carry the fused accumulations
        acc_t = per.tile([P, NCHUNK], F32)
        acc_d = per.tile([P, NCHUNK], F32)
        acc_s = per.tile([P, NCHUNK], F32)
        acc_p2 = per.tile([P, NCHUNK], F32)
        acc_f1 = per.tile([P, NCHUNK], F32)
        acc_fn = per.tile([P, NCHUNK], F32)
        acc_w = per.tile([P, NCHUNK], F32)
        acc_hw = per.tile([P, NCHUNK], F32)
        acc_ss = per.tile([P, 1], F32)
        acc_nn = per.tile([P, 1], F32)

        for i in range(NCHUNK):
            cs = bass.ts(i, FC)
            p = inp.tile([P, FC], F32, tag="p")
            nc.sync.dma_start(p[:], prob[:, cs])
            t = inp.tile([P, FC], F32, tag="t")
            nc.sync.dma_start(t[:], lab[:, cs])
            r1 = inp.tile([P, FC], F32, tag="r1")
            nc.sync.dma_start(r1[:], rlg[:, cs])
            r2 = inp.tile([P, FC], F32, tag="r2")
            nc.sync.dma_start(r2[:], rsp[:, cs])

            # ---- focal / tversky ----
            pc = w32.tile([P, FC], F32, tag="pc")
            nc.vector.tensor_scalar(pc[:], p[:], EPS, 1.0 - EPS, ALU.max, ALU.min)
            d = w32.tile([P, FC], F32, tag="d")
            nc.vector.scalar_tensor_tensor(
                d[:], pc[:], -1.0, t[:], ALU.mult, ALU.add,
                accum_out=acc_d[:, i : i + 1])
            tbf = wbf.tile([P, FC], BF16, tag="tbf")
            nc.vector.tensor_scalar(
                tbf[:], t[:], 1.0, None, ALU.mult, ALU.add,
                accum_out=acc_t[:, i : i + 1])
            u = w32.tile([P, FC], F32, tag="u")
            nc.vector.scalar_tensor_tensor(
                u[:], d[:], -1.0, d[:], ALU.mult, ALU.max)
            s = wbf.tile([P, FC], BF16, tag="s")
            nc.scalar.activation(
                s[:], d[:], ACTF.Square, accum_out=acc_s[:, i : i + 1])
            lg = wbf.tile([P, FC], BF16, tag="lg")
            nc.scalar.activation(lg[:], u[:], ACTF.Ln, bias=1.0, scale=-1.0)
            f1 = wbf.tile([P, FC], BF16, tag="f1")
            nc.vector.scalar_tensor_tensor(
                f1[:], s[:], 1.0, lg[:], ALU.mult, ALU.mult)
            nc.vector.scalar_tensor_tensor(
                fneg[:, cs], tbf[:], 0.5, f1[:], ALU.is_lt, ALU.mult,
                accum_out=acc_fn[:, i : i + 1])
            # sum the *quantized* f1 tile so (sf1 - sfn) is elementwise exact
            fsc = scr.tile([P, FC], BF16, tag="fsc")
            nc.vector.tensor_scalar(
                fsc[:], f1[:], 1.0, None, ALU.mult, ALU.add,
                accum_out=acc_f1[:, i : i + 1])
            ssc = scr.tile([P, FC], BF16, tag="ssc")
            nc.scalar.activation(
                ssc[:], pc[:], ACTF.Square, accum_out=acc_p2[:, i : i + 1])

            # ---- huber + gating ----
            rlt = wbf.tile([P, FC], BF16, tag="rlt")
            nc.scalar.activation(rlt[:], r2[:], ACTF.Ln, bias=1.0, scale=1.0)
            rlb = wbf.tile([P, FC], BF16, tag="rlb")
            nc.gpsimd.tensor_copy(rlb[:], r1[:])
            dh = wbf.tile([P, FC], BF16, tag="dh")
            nc.gpsimd.tensor_tensor(dh[:], rlb[:], rlt[:], ALU.subtract)
            a = wbf.tile([P, FC], BF16, tag="a")
            nc.vector.scalar_tensor_tensor(
                a[:], dh[:], -1.0, dh[:], ALU.mult, ALU.max)
            v = wbf.tile([P, FC], BF16, tag="v")
            nc.vector.tensor_scalar(v[:], a[:], 0.5, -1.0, ALU.min, ALU.add)
            zz = wbf.tile([P, FC], BF16, tag="zz")
            nc.vector.scalar_tensor_tensor(
                zz[:], v[:], 1.0, v[:], ALU.add, ALU.mult)
            hc = wbf.tile([P, FC], BF16, tag="hc")
            nc.gpsimd.tensor_tensor(hc[:], zz[:], a[:], ALU.add)
            zm = wbf.tile([P, FC], BF16, tag="zm")
            nc.vector.scalar_tensor_tensor(
                zm[:], p[:], 10.0, r2[:], ALU.mult, ALU.max)
            zb = wbf.tile([P, FC], BF16, tag="zb")
            nc.vector.tensor_scalar(zb[:], r2[:], 50.0, 3.0, ALU.is_ge, ALU.mult)
            w = wbf.tile([P, FC], BF16, tag="w")
            nc.vector.scalar_tensor_tensor(
                w[:], zm[:], 1.0, zb[:], ALU.is_gt, ALU.add,
                accum_out=acc_w[:, i : i + 1])
            hsc = scr.tile([P, FC], BF16, tag="hsc")
            nc.vector.scalar_tensor_tensor(
                hsc[:], hc[:], 1.0, w[:], ALU.mult, ALU.mult,
                accum_out=acc_hw[:, i : i + 1])

        # ---- n_pos -> subset top-k target ----
        tsum = sml.tile([P, 1], F32, tag="tsum")
        nc.vector.tensor_reduce(tsum[:], acc_t[:], AXX, ALU.add)
        npbc = psp.tile([P, 1], F32, tag="npbc")
        nc.tensor.matmul(npbc[:], ones[:], tsum[:], start=True, stop=True)
        npv = sml.tile([P, 1], F32, tag="npv")
        nc.scalar.activation(npv[:], npbc[:], ACTF.Identity)
        ka = sml.tile([P, 1], F32, tag="ka")
        nc.vector.tensor_scalar(ka[:], npv[:], 10.0 / SUBSTRIDE, None, ALU.mult)
        kb = sml.tile([P, 1], F32, tag="kb")
        nc.vector.tensor_scalar(
            kb[:], npv[:], -1.0 / SUBSTRIDE, float(NPIX // SUBSTRIDE),
            ALU.mult, ALU.add)
        kk = sml.tile([P, 1], F32, tag="kk")
        nc.vector.scalar_tensor_tensor(kk[:], ka[:], 1.0, kb[:], ALU.mult, ALU.min)

        # strided subset of fneg (every 16th element)
        sub = per.tile([P, NSUB], BF16)
        fview = fneg[:].rearrange("p (n s) -> p n s", s=SUBSTRIDE)[:, :, 0:1]
        nc.vector.tensor_copy(sub[:].unsqueeze(-1), fview)

        # ---- binary search for theta (in f1 units, negative) ----
        th = sml.tile([P, 1], F32, tag="th")
        nc.vector.memset(th[:], -2.0)
        delta = 1.0
        for _ in range(NITER):
            csc = sml.tile([P, NSUB], BF16, tag="csc")
            cnt = sml.tile([P, 1], F32, tag="cnt")
            nc.vector.tensor_scalar(
                csc[:], sub[:], th[:], None, ALU.is_lt, ALU.add,
                accum_out=cnt[:])
            cbc = psp.tile([P, 1], F32, tag="cbc")
            nc.tensor.matmul(cbc[:], ones[:], cnt[:], start=True, stop=True)
            sg = sml.tile([P, 1], F32, tag="sg")
            nc.scalar.activation(sg[:], cbc[:], ACTF.Sign, bias=kk[:], scale=-1.0)
            th2 = sml.tile([P, 1], F32, tag="th")
            nc.scalar.activation(th2[:], sg[:], ACTF.Identity, bias=th[:], scale=delta)
            th = th2
            delta *= 0.5

        # ---- exact masked count + sum at theta over the full map ----
        nsc = scr.tile([P, F], BF16, tag="nsc")
        nc.vector.tensor_scalar(
            nsc[:], fneg[:], th[:], None, ALU.is_lt, ALU.add,
            accum_out=acc_nn[:])
        ssc2 = scr.tile([P, F], BF16, tag="nsc")
        nc.vector.scalar_tensor_tensor(
            ssc2[:], fneg[:], th[:], fneg[:], ALU.is_lt, ALU.mult,
            accum_out=acc_ss[:])

        # ---- pack everything into out[1, NOUT] via ones-matmuls ----
        fin = psp.tile([1, NOUT], F32, tag="fin")
        nc.tensor.matmul(fin[:, SL_T:SL_T + 4], ones1[:], acc_t[:], start=True, stop=True)
        nc.tensor.matmul(fin[:, SL_D:SL_D + 4], ones1[:], acc_d[:], start=True, stop=True)
        nc.tensor.matmul(fin[:, SL_S:SL_S + 4], ones1[:], acc_s[:], start=True, stop=True)
        nc.tensor.matmul(fin[:, SL_P2:SL_P2 + 4], ones1[:], acc_p2[:], start=True, stop=True)
        nc.tensor.matmul(fin[:, SL_F1:SL_F1 + 4], ones1[:], acc_f1[:], start=True, stop=True)
        nc.tensor.matmul(fin[:, SL_FN:SL_FN + 4], ones1[:], acc_fn[:], start=True, stop=True)
        nc.tensor.matmul(fin[:, SL_W:SL_W + 4], ones1[:], acc_w[:], start=True, stop=True)
        nc.tensor.matmul(fin[:, SL_HW:SL_HW + 4], ones1[:], acc_hw[:], start=True, stop=True)
        nc.tensor.matmul(fin[:, SL_SS:SL_SS + 1], ones1[:], acc_ss[:], start=True, stop=True)
        nc.tensor.matmul(fin[:, SL_NN:SL_NN + 1], ones1[:], acc_nn[:], start=True, stop=True)
        nc.tensor.matmul(fin[:, SL_TH:SL_TH + 1], ones1[:], th[:], start=True, stop=True)
        nc.tensor.matmul(fin[:, SL_KK:SL_KK + 1], ones1[:], kk[:], start=True, stop=True)

        osb = sml.tile([1, NOUT], F32, tag="osb")
        nc.scalar.activation(osb[:], fin[:], ACTF.Identity)
        nc.sync.dma_start(out[:, :], osb[:])


def build_nc():
    nc = bacc.Bacc(
        "TRN2", target_bir_lowering=False, debug=False,
        enable_asserts=True, num_devices=B)
    prob = nc.dram_tensor("prob", [P, F], F32, kind="ExternalInput").ap()
    lab = nc.dram_tensor("lab", [P, F], F32, kind="ExternalInput").ap()
    rlg = nc.dram_tensor("rlg", [P, F], F32, kind="ExternalInput").ap()
    rsp = nc.dram_tensor("rsp", [P, F], F32, kind="ExternalInput").ap()
    out = nc.dram_tensor("out", [1, NOUT], F32, kind="ExternalOutput").ap()
    with tile.TileContext(nc) as tc:
        _trace_body(tc, out, prob, lab, rlg, rsp)
    nc.compile()
    return nc


_NC = None


def _get_nc():
    global _NC
    if _NC is None:
        _NC = build_nc()
    return _NC


def make_in_maps(prob_map, label_map, rain_logit, rain_spatial_true):
    maps = []
    for b in range(B):
        maps.append({
            "prob": np.ascontiguousarray(prob_map[b].reshape(P, F), dtype=np.float32),
            "lab": np.ascontiguousarray(label_map[b].reshape(P, F), dtype=np.float32),
            "rlg": np.ascontiguousarray(rain_logit[b].reshape(P, F), dtype=np.float32),
            "rsp": np.ascontiguousarray(rain_spatial_true[b].reshape(P, F), dtype=np.float32),
        })
    return maps


def _host_focal_sample(prob, lab, b):
    """Exact (float64) reference focal for one sample - slow fallback."""
    p = np.clip(prob.reshape(-1).astype(np.float64), EPS, 1.0 - EPS)
    t = lab.reshape(-1).astype(np.float64)
    bce = -(2.0 * t * np.log(p) + (1.0 - t) * np.log1p(-p))
    pos = t >= 0.5
    p_t = np.where(pos, p, 1.0 - p)
    a_t = np.where(pos, 0.75, 0.25)
    focal = a_t * (1.0 - p_t) ** 2 * bce
    n_pos = int(pos.sum())
    n_neg = focal.size - n_pos
    if n_pos > 0:
        k = min(10 * n_pos, n_neg)
        negf = focal[~pos]
        top = np.partition(negf, negf.size - k)[negf.size - k:].sum() if k > 0 else 0.0
        return (focal[pos].sum() + top) / max(n_pos + k, 1)
    import jax
    with jax.default_device(jax.devices("cpu")[0]):
        rs = np.asarray(jax.random.uniform(jax.random.key(42), (B, focal.size)))[b]
    order = np.argsort(np.where(pos, np.inf, rs), kind="stable")
    n_s = max(n_neg // 100, 1)
    return focal[order[:n_s]].sum() / n_s


def combine(vecs, prob_map, rain_logit, pred_phys, label_map,
            rain_spatial_true, phys_targets, phys_mu, phys_std):
    fls, tvs = [], []
    reg_num = 0.0
    reg_den = 0.0
    for b in range(B):
        v = vecs[b]
        st = v[SL_T:SL_T + 4].sum()
        sd = v[SL_D:SL_D + 4].sum()
        ss = v[SL_S:SL_S + 4].sum()
        sp2 = v[SL_P2:SL_P2 + 4].sum()
        sf1 = v[SL_F1:SL_F1 + 4].sum()
        sfn = v[SL_FN:SL_FN + 4].sum()
        sw = v[SL_W:SL_W + 4].sum()
        shw = v[SL_HW:SL_HW + 4].sum()
        S, Ncnt = v[SL_SS], v[SL_NN]
        th = v[SL_TH] / P
        n_pos = int(round(st))
        spc = st - sd
        tp = (st + sp2 - ss) / 2.0
        fp = spc - tp
        fn = st - tp
        tvs.append(1.0 - (tp + 1.0) / (tp + 0.3 * fp + 0.7 * fn + 1.0))
        n_neg = NPIX - n_pos
        k = min(10 * n_pos, n_neg)
        ok = n_pos > 0 and k >= 1600 and abs(Ncnt - k) <= max(64.0, 0.02 * k)
        if ok:
            top_f1 = S + (k - Ncnt) * th
            pos_f1 = sf1 - sfn
            fls.append((-1.5 * pos_f1 - 0.25 * top_f1) / max(n_pos + k, 1))
        else:
            fls.append(_host_focal_sample(prob_map[b], label_map[b], b))
        reg_num += 2.0 * shw
        reg_den += sw
    fl = float(np.mean(fls))
    tv = float(np.mean(tvs))
    reg = reg_num / max(reg_den, 1.0)
    tgt = np.nan_to_num(
        (phys_targets.astype(np.float64) - phys_mu.astype(np.float64))
        / (phys_std.astype(np.float64) + 1e-6))
    aux = float(np.mean((pred_phys.astype(np.float64) - tgt) ** 2))
    total = fl + 0.5 * tv + 1.0 * reg + 0.1 * aux
    f = np.float32
    return (f(total), f(fl), f(tv), f(reg), f(aux))


def kernel(prob_map, rain_logit, pred_phys, label_map, rain_max_true,
           rain_spatial_true, phys_targets, phys_mu, phys_std):
    nc = _get_nc()
    in_maps = make_in_maps(prob_map, label_map, rain_logit, rain_spatial_true)
    res = run_bass_kernel_spmd(nc, in_maps, core_ids=list(range(B)))
    vecs = [np.asarray(res.results[b]["out"]).reshape(-1).astype(np.float64)
            for b in range(B)]
    return combine(vecs, prob_map, rain_logit, pred_phys, label_map,
                   rain_spatial_true, phys_targets, phys_mu, phys_std)
